# revision 29
# baseline (speedup 1.0000x reference)
"""Multi-head attention forward on 8 Trainium2 NeuronCores.

Problem: B=4, S=2048, E=1024, H=16, D=64 (fp32 in/out).

Sharding: 8 cores = (batch b, head-half). Each core owns 8 of the 16 heads
for one batch element and computes Q/K/V projections for those heads over
the FULL 2048-row sequence — so the attention phase needs NO K/V exchange
at all (the old seq-sharded design lost ~70us to serialized pair
AllGathers). The only communication is at the O projection: each core
computes the partial y = ctx_half @ Wo_half and the pair combines partials
with 4 pipelined ReduceScatters (one per 512-query chunk, bf16 partials),
which land in the loose back half of the schedule. The host re-interleaves
the scattered rows.

All matmuls run in bf16 (inputs host-cast; fp32 PSUM accumulation). Layouts
avoid all on-chip transposes: x^T is transposed on the HOST, K^T/Q^T kept
in [n, s] form, V in [s, n] form with a ones column per head (softmax
denominator from the same matmul as attn@V), scores built transposed
([k, q]).

Schedule: single software-pipelined pass. Projections (K/Q per head pair,
V per key block) feed 16 attention windows (4 head pairs x 4 query chunks).
The Scalar-engine exp stream (~263us) is the per-window floor; projections
and O-projection blocks are interleaved between windows so the PE never
starves. attn tiles are per-key-block so consecutive windows pipeline
without a barrier.
"""

import os
import sys
import types

import numpy as np

sys.path.insert(0, "/opt/trn_rl_repo")

B, S, E, H = 4, 2048, 1024, 16
D = E // H          # 64
NCORES = 8
NH = H // 2         # heads per core = 8
NP = NH // 2        # head pairs per core = 4
NE = E // 2         # output features per core for QKV = 512

_compiled = None


def _install_prof_hook():
    try:
        import antenv.axon_hooks  # noqa: F401
        return
    except ImportError:
        pass
    try:
        import antenv
        from trn_agent_boot.trn_boot import _ntff_profile_via_ctypes
    except ImportError:
        return
    mod = types.ModuleType("antenv.axon_hooks")
    mod._hook = None
    mod.set_axon_ntff_profile_hook = lambda h: setattr(mod, "_hook", h)
    mod.get_axon_ntff_profile_hook = lambda: mod._hook
    sys.modules["antenv.axon_hooks"] = mod
    antenv.axon_hooks = mod
    try:
        mod._hook = _ntff_profile_via_ctypes("/opt/axon/libaxon_pjrt.so")
    except Exception:
        mod._hook = None


def _build():
    from contextlib import ExitStack

    from concourse import bacc
    import concourse.mybir as mybir
    from concourse import tile_utils
    from concourse.tile import TileContext

    tile_utils.max_sbuf_usage = 207 * 1024

    F32 = mybir.dt.float32
    BF16 = mybir.dt.bfloat16
    Exp = mybir.ActivationFunctionType.Exp
    Add = mybir.AluOpType.add
    Bypass = mybir.AluOpType.bypass

    nc = bacc.Bacc("TRN2", target_bir_lowering=False, debug=False)

    xb = nc.dram_tensor("xb", [E, S], BF16, kind="ExternalInput")
    # weights host-prepacked (per-core head-half slices):
    #   wq/wk [nb=4, p, eb=8, nl=128], wv [p, eb=8, 512], wo [nc2=2, p, eb=4, 512]
    wq = nc.dram_tensor("wq", [NP, 128, 8, 128], BF16, kind="ExternalInput")
    wk = nc.dram_tensor("wk", [NP, 128, 8, 128], BF16, kind="ExternalInput")
    wv = nc.dram_tensor("wv", [128, 8, NE], BF16, kind="ExternalInput")
    wo = nc.dram_tensor("wo", [2, 128, 4, 512], BF16, kind="ExternalInput")
    # y rows, per query chunk qc: the pair-slot's 256 reduced rows
    y = nc.dram_tensor("y", [4 * 256, E], F32, kind="ExternalOutput")

    y_v = y.ap().rearrange("(r p) e -> r p e", p=128)   # [8, 128, E]

    EB = E // 128        # 8 e-chunks (contraction blocks for QKV)
    KB = S // 128        # 16 key blocks
    SC = S // 512        # 4 s/query chunks
    PAIRS = [[2 * i, 2 * i + 1] for i in range(4)]
    inv_sqrt_d = 1.0 / float(np.sqrt(D))

    with TileContext(nc) as tc:
        with ExitStack() as es:
            xTp = es.enter_context(tc.tile_pool(name="xT", bufs=1))
            kTp = es.enter_context(tc.tile_pool(name="kT", bufs=1))
            qTp = es.enter_context(tc.tile_pool(name="qT", bufs=1))
            vp = es.enter_context(tc.tile_pool(name="vA", bufs=1))
            ctxp = es.enter_context(tc.tile_pool(name="ctx", bufs=1))
            attnp = es.enter_context(tc.tile_pool(name="attn", bufs=16))
            wkqp = es.enter_context(tc.tile_pool(name="wkq", bufs=8))
            wvp = es.enter_context(tc.tile_pool(name="wv", bufs=1))
            wop = es.enter_context(tc.tile_pool(name="wo", bufs=2))
            stp = es.enter_context(tc.tile_pool(name="st", bufs=4))
            drp = es.enter_context(tc.tile_pool(name="dr", bufs=2, space="DRAM"))
            gop = es.enter_context(tc.tile_pool(name="go", bufs=2, space="DRAM"))
            ybp = es.enter_context(tc.tile_pool(name="yb", bufs=2))
            ytp = es.enter_context(tc.tile_pool(name="yt", bufs=2))
            nrmp = es.enter_context(tc.tile_pool(name="nrm", bufs=2))
            stgp = es.enter_context(tc.tile_pool(name="stg", bufs=2))
            psP = es.enter_context(tc.tile_pool(name="psP", bufs=2, space="PSUM"))
            psS = es.enter_context(tc.tile_pool(name="psS", bufs=2, space="PSUM"))
            psC = es.enter_context(tc.tile_pool(name="psC", bufs=2, space="PSUM"))

            xT = xTp.tile([128, EB, S], BF16)        # x^T  [e, s] full batch row
            kT = kTp.tile([128, NP, S], BF16)        # K^T  [n, s] local heads
            qT = qTp.tile([128, NP, S], BF16)        # Q^T  [n, q] local heads
            # V with a ones column per head (softmax denominator row)
            vA = vp.tile([128, KB, NH, D + 1], BF16)
            ctx = ctxp.tile([128, NP, S], BF16)      # ctx^T [e_local, q]

            wkc = [None] * NP
            wqc = [None] * NP

            def load_wk(nb, eng=None):
                t = wkqp.tile([128, EB, 128], BF16, tag="wkq", name=f"wk{nb}")
                (eng or nc.gpsimd).dma_start(t[:], wk.ap()[nb])
                wkc[nb] = t

            def load_wq(nb, eng=None):
                t = wkqp.tile([128, EB, 128], BF16, tag="wkq", name=f"wq{nb}")
                (eng or nc.gpsimd).dma_start(t[:], wq.ap()[nb])
                wqc[nb] = t

            def k_proj(nb, scs):
                for sc in scs:
                    ps = psP.tile([128, 512], F32, tag="pp", name=f"pk{nb}_{sc}")
                    for eb in range(EB):
                        nc.tensor.matmul(ps[:], wkc[nb][:, eb, :],
                                         xT[:, eb, sc * 512:(sc + 1) * 512],
                                         start=(eb == 0), stop=(eb == EB - 1))
                    nc.vector.tensor_copy(
                        kT[:, nb, sc * 512:(sc + 1) * 512], ps[:])

            def q_proj(nb, scs):
                for sc in scs:
                    ps = psP.tile([128, 512], F32, tag="pp", name=f"pq{nb}_{sc}")
                    for eb in range(EB):
                        nc.tensor.matmul(ps[:], wqc[nb][:, eb, :],
                                         xT[:, eb, sc * 512:(sc + 1) * 512],
                                         start=(eb == 0), stop=(eb == EB - 1))
                    nc.vector.tensor_copy(
                        qT[:, nb, sc * 512:(sc + 1) * 512], ps[:])

            wvc = wvp.tile([128, EB, NE], BF16, tag="wv", name="wv")

            def v_proj(sbs):
                for sb in sbs:
                    ps = psP.tile([128, 512], F32, tag="pp", name=f"pv{sb}")
                    for eb in range(EB):
                        nc.tensor.matmul(ps[:],
                                         xT[:, eb, sb * 128:(sb + 1) * 128],
                                         wvc[:, eb, :],
                                         start=(eb == 0), stop=(eb == EB - 1))
                    # [p, (h d)] -> vA[:, sb, h, 0:D]
                    nc.vector.tensor_copy(
                        vA[:, sb, :, 0:D],
                        ps.rearrange("p (h d) -> p h d", d=D))

            def attn_window(j, q0, qw):
                # scores^T + exp + attn@V for head pair j, queries [q0,q0+qw).
                # attn tiles are per-kb so window boundaries pipeline.
                qs = slice(q0, q0 + qw)
                ats = []
                for kb in range(KB):
                    # allocate the full 2-bank tile even for narrow windows
                    # so each head's accumulation group owns its own bank
                    sps = psS.tile([128, 2, 512], F32, tag="sps",
                                   name=f"sc{j}_{q0}_{kb}")
                    for hh in range(2):
                        p0 = hh * 64
                        nc.tensor.matmul(
                            sps[:, hh, 0:qw],
                            kT[p0:p0 + 64, j, kb * 128:(kb + 1) * 128],
                            qT[p0:p0 + 64, j, qs],
                            start=True, stop=True)
                    at = attnp.tile([128, 2, qw], BF16, tag="attn",
                                    name=f"at{j}_{q0}_{kb}")
                    nc.scalar.activation(
                        at[:], sps[:, :, 0:qw], Exp,
                        scale=inv_sqrt_d)
                    ats.append(at)

                cpss = [psC.tile([128, qw], F32, tag="cps",
                                 name=f"cp{j}_{q0}_{i}") for i in range(2)]
                for kb in range(KB):
                    for hh in range(2):
                        nc.tensor.matmul(
                            cpss[hh][0:D + 1, :],
                            vA[:, kb, 2 * j + hh, :],
                            ats[kb][:, hh, :],
                            start=(kb == 0), stop=(kb == KB - 1))
                for hh in range(2):
                    # stage PSUM -> SBUF so the cps banks recycle early
                    cpb = nrmp.tile([D + 1, qw], F32, tag="cpb")
                    nc.vector.tensor_copy(cpb[:], cpss[hh][0:D + 1, :])
                    den = nrmp.tile([1, qw], F32, tag="den")
                    nc.vector.tensor_copy(den[:], cpb[D:D + 1, :])
                    nc.vector.reciprocal_approx_fast(den[:], den[:])
                    bcast = nrmp.tile([64, qw], F32, tag="bc")
                    nc.gpsimd.partition_broadcast(bcast[:], den[:])
                    if hh == 0:
                        nc.vector.tensor_mul(
                            ctx[0:64, j, qs], cpb[0:D, :], bcast[:])
                    else:
                        stg = stgp.tile([64, qw], BF16, tag="stg")
                        nc.vector.tensor_mul(stg[:], cpb[0:D, :], bcast[:])
                        nc.sync.dma_start(ctx[64:128, j, qs], stg[:])

            woc = [None, None]

            def load_wo(nc2):
                t = wop.tile([128, 4, 512], BF16, tag="wo", name=f"wo{nc2}")
                nc.gpsimd.dma_start(t[:], wo.ap()[nc2])
                woc[nc2] = t

            # exchange chunks: (q0, qw). Last sweep is split into two
            # 256-row half-chunks so the final (serial, exposed) pair
            # exchange is half-size and the second-to-last hides under the
            # last half-sweep.
            CHUNKS = [(0, 512), (512, 512), (1024, 512), (1536, 256),
                      (1792, 256)]
            # y row-block (of 128) base per chunk
            YB0 = [0, 2, 4, 6, 7]
            pins = [None] * len(CHUNKS)
            pouts = [None] * len(CHUNKS)

            def o_block(ch, nc2, qb):
                # partial y rows [q0+qb*128 ...] x e-out cols [nc2*512..]
                # pin layout [slot, qb-in-slot, p, e]: slot s holds the rows
                # destined for pair-rank s
                q0, qw = CHUNKS[ch]
                nqb = qw // 256          # 128-row blocks per rank-slot
                if pins[ch] is None:
                    pins[ch] = drp.tile([2, nqb, 128, E], BF16, tag="pin",
                                        name=f"pin{ch}")
                ps = psP.tile([128, 512], F32, tag="pp", name=f"py{ch}_{nc2}_{qb}")
                qq = q0 + qb * 128
                for eb in range(NP):
                    nc.tensor.matmul(ps[:],
                                     ctx[:, eb, qq:qq + 128],
                                     woc[nc2][:, eb, :],
                                     start=(eb == 0), stop=(eb == NP - 1))
                st = stp.tile([128, 512], BF16, tag="st", name=f"yst{ch}_{nc2}_{qb}")
                nc.vector.tensor_copy(st[:], ps[:])
                nc.sync.dma_start(
                    pins[ch][qb // nqb, qb % nqb, :, nc2 * 512:(nc2 + 1) * 512],
                    st[:])

            def xc_chunk(ch):
                # pair ReduceScatter of the partial chunk; each slot receives
                # its half of the reduced rows
                nqb = CHUNKS[ch][1] // 256
                pouts[ch] = gop.tile([nqb, 128, E], BF16, tag="po",
                                     name=f"po{ch}")
                nc.gpsimd.collective_compute(
                    "ReduceScatter", Add, replica_groups=PAIRS,
                    ins=[pins[ch].opt()], outs=[pouts[ch].opt()])

            def y_out(ch):
                nqb = CHUNKS[ch][1] // 256
                for qb in range(nqb):
                    yb = ybp.tile([128, E], BF16, tag="yb")
                    nc.scalar.dma_start(yb[:], pouts[ch][qb])
                    yt = ytp.tile([128, E], F32, tag="yt")
                    nc.vector.tensor_copy(yt[:], yb[:])
                    nc.sync.dma_start(y_v[YB0[ch] + qb], yt[:])

            # ---------------- emission (priority) order ----------------
            load_wk(0, nc.sync); load_wq(0)
            # x arrives host-transposed [E, S]; [128,512] chunks spread over
            # three queues, s-chunk-major so the first projections start early
            xbT_v = xb.ap().rearrange("(eb p) q -> p eb q", p=128)
            for sc in range(SC - 1):
                for eb in range(EB):
                    eng = nc.sync if eb % 2 == 0 else nc.scalar
                    eng.dma_start(
                        xT[:, eb, sc * 512:(sc + 1) * 512],
                        xbT_v[:, eb, sc * 512:(sc + 1) * 512])
            sc = SC - 1
            for eb in range(EB):
                nc.gpsimd.dma_start(
                    xT[:, eb, sc * 512:(sc + 1) * 512],
                    xbT_v[:, eb, sc * 512:(sc + 1) * 512])
            nc.gpsimd.memset(vA[:, :, :, D], 1.0)    # ones column (all heads)
            nc.gpsimd.dma_start(wvc[:], wv.ap())
            load_wk(1); load_wq(1)
            load_wk(2); load_wq(2)
            load_wk(3); load_wq(3)
            load_wo(0); load_wo(1)

            # sweep 0: projections + first windows. Q columns for sweep sc
            # are deferred to sweep sc-1 as PE filler (only K and V must be
            # complete before a pair's first window).
            k_proj(0, range(SC))
            q_proj(0, [0])
            # window 0 scores can start right after k/q(0); V-proj fills the
            # PE while window 0's exp stream runs
            v_proj(range(KB))
            q_proj(0, [1])
            attn_window(0, 0, 512)
            k_proj(1, range(SC)); q_proj(1, [0, 1])
            attn_window(1, 0, 512)
            k_proj(2, range(SC)); q_proj(2, [0, 1])
            attn_window(2, 0, 512)
            k_proj(3, range(SC)); q_proj(3, [0, 1])
            attn_window(3, 0, 512)
            # sweeps 1-2: o-blocks of the previous chunk spread 3/2/2/1 as
            # PE filler (exchange lag margins are huge here); deferred Q
            # projections fill the back windows. y unload trails its
            # exchange by >1 sweep so its DMAs never block a queue.
            for si, qc in ((0, 1), (1, 2)):
                ch = si
                attn_window(0, qc * 512, 512)
                o_block(ch, 0, 0); o_block(ch, 0, 1); o_block(ch, 0, 2)
                attn_window(1, qc * 512, 512)
                o_block(ch, 0, 3); o_block(ch, 1, 0)
                attn_window(2, qc * 512, 512)
                o_block(ch, 1, 1); o_block(ch, 1, 2)
                q_proj(0, [qc + 1]); q_proj(1, [qc + 1])
                attn_window(3, qc * 512, 512)
                o_block(ch, 1, 3)
                xc_chunk(ch)
                q_proj(2, [qc + 1]); q_proj(3, [qc + 1])
                if si == 1:
                    y_out(0)
            # sweep 3a (queries 1536:1792): chunk-2 o-blocks front-loaded so
            # its exchange hides under sweep 3b
            attn_window(0, 1536, 256)
            for qb in range(4):
                o_block(2, 0, qb)
            attn_window(1, 1536, 256)
            for qb in range(4):
                o_block(2, 1, qb)
            xc_chunk(2)
            attn_window(2, 1536, 256)
            y_out(1)
            attn_window(3, 1536, 256)
            # sweep 3b (queries 1792:2048): chunk-3 (=sweep 3a rows)
            # o-blocks + exchange hide under these windows
            attn_window(0, 1792, 256)
            o_block(3, 0, 0); o_block(3, 0, 1)
            attn_window(1, 1792, 256)
            o_block(3, 1, 0)
            attn_window(2, 1792, 256)
            o_block(3, 1, 1)
            xc_chunk(3)
            y_out(2)
            attn_window(3, 1792, 256)
            # tail: only the last 256-row half-chunk is exposed
            o_block(4, 0, 0); o_block(4, 0, 1)
            o_block(4, 1, 0); o_block(4, 1, 1)
            xc_chunk(4)
            y_out(3)
            y_out(4)

    nc.compile()
    return nc


def kernel(x, Wq, Wk, Wv, Wo):
    global _compiled
    _install_prof_hook()
    import ml_dtypes
    from concourse import bass_utils

    if _compiled is None:
        _compiled = _build()
    nc = _compiled

    bf16 = ml_dtypes.bfloat16
    x = np.ascontiguousarray(x, dtype=np.float32)
    Wq = np.asarray(Wq, dtype=np.float32).astype(bf16)
    Wk = np.asarray(Wk, dtype=np.float32).astype(bf16)
    Wv = np.asarray(Wv, dtype=np.float32).astype(bf16)
    Wo = np.asarray(Wo, dtype=np.float32).astype(bf16)

    def pack_col(W, half):
        # [E, n-half] -> [nb, p, eb, 128] contiguous chunks
        a = W[:, half * NE:(half + 1) * NE]
        a = a.reshape(8, 128, NP, 128).transpose(2, 1, 0, 3)
        return np.ascontiguousarray(a)

    def pack_v(W, half):
        a = W[:, half * NE:(half + 1) * NE]
        a = a.reshape(8, 128, NE).transpose(1, 0, 2)
        return np.ascontiguousarray(a)

    def pack_o(W, half):
        # [E-half rows, E] -> [nc2, p, eb=4, 512]
        a = W[half * NE:(half + 1) * NE, :]
        a = a.reshape(4, 128, 2, 512).transpose(2, 1, 0, 3)
        return np.ascontiguousarray(a)

    packed = {}
    for half in range(2):
        packed[half] = {
            "wq": pack_col(Wq, half),
            "wk": pack_col(Wk, half),
            "wv": pack_v(Wv, half),
            "wo": pack_o(Wo, half),
        }

    in_maps = []
    for c in range(NCORES):
        b, half = c // 2, c % 2
        in_maps.append({
            "xb": np.ascontiguousarray(x[b].astype(bf16).T),
            **packed[half],
        })

    trace = bool(int(os.environ.get("KERNEL_TRACE", "0")))
    res = bass_utils.run_bass_kernel_spmd(
        nc, in_maps, core_ids=list(range(NCORES)), trace=trace)
    kernel.last_result = res

    chunks = [(0, 512), (512, 512), (1024, 512), (1536, 256), (1792, 256)]
    out = np.empty((B, S, E), dtype=np.float32)
    for c in range(NCORES):
        b, half = c // 2, c % 2
        yc = res.results[c]["y"]
        yr = 0
        for q0, qw in chunks:
            nr = qw // 2
            r0 = q0 + half * nr
            out[b, r0:r0 + nr] = yc[yr:yr + nr]
            yr += nr
    return out


kernel.last_result = None
